# revision 17
# baseline (speedup 1.0000x reference)
"""Affinity-propagate (SPN) Trainium2 Bass kernel, fp16 pipeline.

Computation (per batch element, see reference):
    w = g / conv3x3_ones(|g|)          # gates, [8, H, W], computed once
    d_{k+1} = max_c conv3x3_ones(w_c * d_k)   # 8 iterations

Distribution: pure data parallel, batch element b -> NeuronCore b (8 cores).

Per-core mapping (H=352 rows as 3 overlapping 128-row tiles):
  - All gate/depth data is fp16: DVE tensor_tensor ops run in 2x_1p mode
    (2 elem/cycle), halving the vector-engine time vs fp32.
  - p = w * d is ONE DVE mult per tile ([128, 8, WB], d broadcast over the
    channel axis; innermost stride 1 keeps the 2x mode).
  - 3x3 conv = tri-band matmul over the H/partition axis (fp16 stationary)
    x 3 PSUM-accumulated W-shifts; W is chunked 3x406 so each channel's
    conv occupies 3 PSUM banks and is evacuated PSUM->SBUF fp16 by ONE
    ScalarE copy (multi-bank AP), amortizing the per-op overhead.
  - channel max: 7 DVE tensor_max ops (fp16 2x), last one writes d in place.
  - seam rows between H tiles are fixed with 1-row SBUF->SBUF DMAs.
  - input g loads are spread across the Sync/Scalar/Vector DMA queues and
    staged 6 deep so the load pipeline stays ahead of gate normalization.
"""
from contextlib import ExitStack

import numpy as np

import concourse.bacc as bacc
import concourse.mybir as mybir
import concourse.tile as tile
from concourse.bass_utils import run_bass_kernel_spmd

F32 = mybir.dt.float32
F16 = mybir.dt.float16

B, C, H, W = 8, 8, 352, 1216
NCHUNK = 3
CW = 406                        # chunk width; 3 chunks of 406 = 1218 >= W
WB = NCHUNK * CW + 2            # 1220: [0]=pad, 1..1216 data, 1217+ pad
N_ITERS = 8
N_CORES = 8

ROW_BASE = [0, 126, 252]       # first global row of each H tile
ROWS = [128, 128, 100]         # partitions used by each H tile


def _build_nc():
    nc = bacc.Bacc("TRN2", target_bir_lowering=False, debug=False,
                   num_devices=N_CORES)
    g = nc.dram_tensor("g", [C, H, W], F32, kind="ExternalInput").ap()
    d_in = nc.dram_tensor("d", [H, W], F32, kind="ExternalInput").ap()
    band = nc.dram_tensor("band", [128, 128], F16, kind="ExternalInput").ap()
    out = nc.dram_tensor("out", [H, W], F32, kind="ExternalOutput").ap()

    with tile.TileContext(nc) as tc, ExitStack() as ctx:
        pw = ctx.enter_context(tc.tile_pool(name="w", bufs=1))
        pd = ctx.enter_context(tc.tile_pool(name="d", bufs=1))
        pc = ctx.enter_context(tc.tile_pool(name="const", bufs=1))
        pg = ctx.enter_context(tc.tile_pool(name="g32", bufs=6))
        pa = ctx.enter_context(tc.tile_pool(name="abs16", bufs=2))
        pr = ctx.enter_context(tc.tile_pool(name="r32", bufs=2))
        pp = ctx.enter_context(tc.tile_pool(name="p", bufs=6))
        pprop = ctx.enter_context(tc.tile_pool(name="prop", bufs=12))
        prm = ctx.enter_context(tc.tile_pool(name="rm", bufs=6))
        psum = ctx.enter_context(tc.tile_pool(name="psum", bufs=2,
                                              space="PSUM"))

        A = pc.tile([128, 128], F16, tag="band", name="bandt")
        nc.sync.dma_start(A[:], band[:])

        wt = [pw.tile([128, C, WB], F16, tag=f"w{t}", name=f"w{t}")
              for t in range(3)]
        dt_ = [pd.tile([128, WB], F16, tag=f"d{t}", name=f"d{t}")
               for t in range(3)]

        # ---- load depth (fp32 staging -> fp16), zero pads ----
        for t in range(3):
            R, rb = ROWS[t], ROW_BASE[t]
            nc.vector.memset(wt[t][:, :, 0:1], 0.0)
            nc.vector.memset(wt[t][:, :, W + 1:WB], 0.0)
            nc.vector.memset(dt_[t][:, 0:1], 0.0)
            nc.vector.memset(dt_[t][:, W + 1:WB], 0.0)
            d32 = pg.tile([128, W], F32, tag="g32", name="d32")
            nc.sync.dma_start(d32[0:R, :], d_in[rb:rb + R, :])
            nc.vector.tensor_copy(dt_[t][0:R, 1:W + 1], d32[0:R, :])

        # ---- phase 0: w = g / conv3x3_ones(|g|) ----
        # channel-PAIR-major order: each pair finishes (incl. its w seam
        # rows) early. Engine queues execute in EMISSION order, so
        # iteration-1 work for pair j is emitted interleaved right after
        # phase-0 pair j+2, filling the PE bubbles of phase 0.
        dma_engines = [nc.sync, nc.scalar]
        a16_bufs = [pa.tile([128, WB], F16, tag="a16", name=f"a16_{i}")
                    for i in range(2)]
        for buf in a16_bufs:
            nc.vector.memset(buf[:, 0:1], 0.0)
            nc.vector.memset(buf[:, W + 1:WB], 0.0)
        a16_ctr = [0]

        def phase0_pair(pair):
            for t in range(3):
                R, rb = ROWS[t], ROW_BASE[t]
                for c in (2 * pair, 2 * pair + 1):
                    g32 = pg.tile([128, W], F32, tag="g32", name="g32")
                    dma_engines[(t * C + c) % 2].dma_start(
                        g32[0:R, :], g[c, rb:rb + R, :])
                    a16 = a16_bufs[a16_ctr[0] % 2]
                    a16_ctr[0] += 1
                    nc.scalar.activation(a16[0:R, 1:W + 1], g32[0:R, :],
                                         mybir.ActivationFunctionType.Abs)
                    ps = psum.tile([128, NCHUNK, 512], F32, tag="ps",
                                   name="ps")
                    for k in range(NCHUNK):
                        for s in range(3):
                            nc.tensor.matmul(
                                ps[0:R, k, 0:CW], A[0:R, 0:R],
                                a16[0:R, k * CW + s:k * CW + s + CW],
                                start=(s == 0), stop=(s == 2))
                    r32 = pr.tile([128, NCHUNK, CW], F32, tag="r32",
                                  name="r32")
                    nc.vector.reciprocal_approx_fast(
                        out=r32[0:R, :, :], in_=ps[0:R, :, 0:CW])
                    rflat = r32[0:R, :, :].rearrange(
                        "p a b -> p (a b)")[:, 0:W]
                    # GpSimd mult (fp32 in, fp16 out) keeps the DVE free
                    # for the interleaved iteration-1 work
                    nc.gpsimd.tensor_mul(wt[t][0:R, c, 1:W + 1],
                                         g32[0:R, :], rflat)
            # w seam rows for this channel pair
            c0, c1 = 2 * pair, 2 * pair + 2
            nc.sync.dma_start(wt[0][127:128, c0:c1, 1:W + 1],
                              wt[1][1:2, c0:c1, 1:W + 1])
            nc.sync.dma_start(wt[1][0:1, c0:c1, 1:W + 1],
                              wt[0][126:127, c0:c1, 1:W + 1])
            nc.sync.dma_start(wt[1][127:128, c0:c1, 1:W + 1],
                              wt[2][1:2, c0:c1, 1:W + 1])
            nc.sync.dma_start(wt[2][0:1, c0:c1, 1:W + 1],
                              wt[1][126:127, c0:c1, 1:W + 1])

        # ---- phase 1 unit: one (tile, channel-pair) of one iteration ----
        # p-pair mult -> 2x (conv matmuls + PSUM->SBUF evac) -> pair max.
        # rm_state[t] holds the running max tile; the last pair writes d.
        rm_state = {}

        def iter_tile_pair(t, pair):
            R = ROWS[t]
            c0 = 2 * pair
            p16 = pp.tile([128, 2, WB], F16, tag="p", name="p16")
            dbc2 = dt_[t][0:R, :].unsqueeze(1).broadcast_to([R, 2, WB])
            nc.vector.tensor_mul(p16[0:R, :, :],
                                 wt[t][0:R, c0:c0 + 2, :], dbc2)
            props = []
            for ci in (0, 1):
                ps = psum.tile([128, NCHUNK, 512], F32, tag="ps", name="ps")
                for kk in range(NCHUNK):
                    for s in range(3):
                        nc.tensor.matmul(
                            ps[0:R, kk, 0:CW], A[0:R, 0:R],
                            p16[0:R, ci, kk * CW + s:kk * CW + s + CW],
                            start=(s == 0), stop=(s == 2))
                prop = pprop.tile([128, NCHUNK * CW], F16, tag="prop",
                                  name="prop")
                nc.scalar.copy(
                    prop[0:R, :].rearrange("p (a b) -> p a b", a=NCHUNK),
                    ps[0:R, :, 0:CW])
                props.append(prop)
            if pair == 0:
                rm = prm.tile([128, W], F16, tag="rm", name="rm")
                nc.vector.tensor_max(rm[0:R, :], props[0][0:R, 0:W],
                                     props[1][0:R, 0:W])
                rm_state[t] = rm
            else:
                pm = prm.tile([128, W], F16, tag="rm", name="pm")
                nc.vector.tensor_max(pm[0:R, :], props[0][0:R, 0:W],
                                     props[1][0:R, 0:W])
                if pair < C // 2 - 1:
                    nc.vector.tensor_max(rm_state[t][0:R, :],
                                         rm_state[t][0:R, :], pm[0:R, :])
                else:
                    nc.vector.tensor_max(dt_[t][0:R, 1:W + 1],
                                         rm_state[t][0:R, :], pm[0:R, :])

        def d_seams():
            nc.sync.dma_start(dt_[0][127:128, 1:W + 1], dt_[1][1:2, 1:W + 1])
            nc.sync.dma_start(dt_[1][0:1, 1:W + 1], dt_[0][126:127, 1:W + 1])
            nc.sync.dma_start(dt_[1][127:128, 1:W + 1], dt_[2][1:2, 1:W + 1])
            nc.sync.dma_start(dt_[2][0:1, 1:W + 1], dt_[1][126:127, 1:W + 1])

        # ---- emission schedule ----
        # phase 0 pairs 0..3 with iteration-1 pair-groups woven in two
        # pairs behind (their w data + seams are complete by then).
        phase0_pair(0)
        phase0_pair(1)
        phase0_pair(2)
        for t in range(3):
            iter_tile_pair(t, 0)
        phase0_pair(3)
        for t in range(3):
            iter_tile_pair(t, 1)
        for pair in (2, 3):
            for t in range(3):
                iter_tile_pair(t, pair)
        d_seams()

        # iterations 2..8, tile-major
        for k in range(1, N_ITERS):
            for t in range(3):
                for pair in range(C // 2):
                    iter_tile_pair(t, pair)
            d_seams()

        # ---- output: fp16 -> fp32 staging -> HBM ----
        outspec = [(0, 0, 128), (1, 2, 128), (2, 2, 100)]
        for t, r0, r1 in outspec:
            o32 = pg.tile([128, W], F32, tag="g32", name="o32")
            nc.scalar.copy(o32[0:ROWS[t], :], dt_[t][0:ROWS[t], 1:W + 1])
            gb = ROW_BASE[t] + r0
            nc.sync.dma_start(out[gb:gb + (r1 - r0), :], o32[r0:r1, :])

    nc.compile()
    return nc


def _band_matrix():
    a = np.zeros((128, 128), dtype=np.float16)
    idx = np.arange(128)
    a[idx, idx] = 1.0
    a[idx[:-1], idx[:-1] + 1] = 1.0
    a[idx[1:], idx[1:] - 1] = 1.0
    return a


_NC_CACHE = None


def kernel(guidance: np.ndarray, blur_depth: np.ndarray) -> np.ndarray:
    """Full inputs in, full output out. Shards batch across 8 NeuronCores."""
    global _NC_CACHE
    guidance = np.asarray(guidance, dtype=np.float32)
    blur_depth = np.asarray(blur_depth, dtype=np.float32)
    assert guidance.shape == (B, C, H, W), guidance.shape
    assert blur_depth.shape == (B, 1, H, W), blur_depth.shape
    if _NC_CACHE is None:
        _NC_CACHE = _build_nc()
    nc = _NC_CACHE
    band = _band_matrix()
    in_maps = [
        {
            "g": np.ascontiguousarray(guidance[b], dtype=np.float32),
            "d": np.ascontiguousarray(blur_depth[b, 0], dtype=np.float32),
            "band": band,
        }
        for b in range(B)
    ]
    res = run_bass_kernel_spmd(nc, in_maps, core_ids=list(range(N_CORES)))
    out = np.stack([res.results[b]["out"] for b in range(B)])[:, None]
    return out.astype(np.float32)
